# revision 17
# baseline (speedup 1.0000x reference)
"""MHA kernel for trn2: 8 cores = 2 (batch DP) x 4 (head TP, 4 heads/core).

Layout strategy (all device tensors d-major / transposed so no on-device
transposes are ever needed); all matmul operands in bf16 (PE runs 1
cycle/row vs 2 for fp32 HIGH mode), accumulation stays fp32 in PSUM:
  - x^T [C, T] per batch (host-transposed, partition-relayout, bf16)
  - Q^T/K^T computed as [d, t] via lhsT=W-slice, rhs=x^T; RoPE via
    multiply-by-signed-sin then PE half-swap permutation matmul (all DVE
    ops same-dtype: bf16*bf16 or f32+f32)
  - V computed as [t, d] bf16 (plus ones column for softmax denominators)
  - S^T duos [tk=128, 2*512]; 2-head packing (K=64, base partitions
    0/64); causal block skipping + diagonal S/exp trimming; exp (bf16
    out) per duo; software-pipelined AV one duo behind S
  - AV accumulates O_aug^T [65, tq] per head; row 64 = softmax denom
  - denominators DMA'd from PSUM rows, reciprocal on DVE (keeps ACT on
    the Exp table only - no table thrash), PE broadcast matmul, per-pair
    normalize directly from the AV PSUM tiles
  - AllGather y^T per (chunk, pair) [128, 512] bf16 (8 small CC ops)
    overlapped with the next pair/chunk; projection per chunk one step
    behind; the final chunk's projection reads even k-tiles (gathered
    first) before odd ones to shorten the serial tail
Host reassembles: concat cout slices, transpose, stack batches.
"""

import sys

sys.path.insert(0, "/opt/trn_rl_repo")

from contextlib import ExitStack  # noqa: E402

import numpy as np  # noqa: E402
import ml_dtypes  # noqa: E402

import concourse.bacc as bacc  # noqa: E402
import concourse.bass as bass  # noqa: E402
import concourse.tile as tile  # noqa: E402
from concourse import mybir  # noqa: E402
from concourse.bass_utils import run_bass_kernel_spmd  # noqa: E402

B, T, C, H = 2, 2048, 1024, 16
HD, HD2 = 64, 32
NCORES, GROUPS, HPG, NPAIRS = 8, 4, 4, 2
TK, TQ = 128, 512
NQ = T // TQ  # 4 q-chunks
NKT = T // TK  # 16 tk tiles
KT = C // 128  # 8 contraction tiles
DGRP = 256  # head dims per core (4 heads * 64)

BF = mybir.dt.bfloat16
F32 = mybir.dt.float32
F32R = mybir.dt.float32r
AF = mybir.ActivationFunctionType
ALU = mybir.AluOpType
SCALE = 1.0 / 8.0  # 1/sqrt(HD)
BF_NP = ml_dtypes.bfloat16


def r32(ap):
    return ap.bitcast(F32R)


def build_nc():
    nc = bacc.Bacc(target_bir_lowering=False)

    xr = nc.dram_tensor("xr", [128, NQ * KT * TQ], BF, kind="ExternalInput")
    wqr = nc.dram_tensor("wqr", [128, KT * DGRP], BF, kind="ExternalInput")
    wkr = nc.dram_tensor("wkr", [128, KT * DGRP], BF, kind="ExternalInput")
    wvr = nc.dram_tensor("wvr", [128, KT * DGRP], BF, kind="ExternalInput")
    wor = nc.dram_tensor("wor", [128, KT * DGRP], BF, kind="ExternalInput")
    cos4 = nc.dram_tensor("cos4", [128, T], BF, kind="ExternalInput")
    sin4 = nc.dram_tensor("sin4", [128, T], BF, kind="ExternalInput")
    perm = nc.dram_tensor("perm", [128, 128], BF, kind="ExternalInput")
    tri = nc.dram_tensor("tri", [TK, TK], BF, kind="ExternalInput")
    bq = nc.dram_tensor("bq", [DGRP, 1], F32, kind="ExternalInput")
    bk = nc.dram_tensor("bk", [DGRP, 1], F32, kind="ExternalInput")
    bo = nc.dram_tensor("bo", [DGRP, 1], F32, kind="ExternalInput")
    sel = nc.dram_tensor("sel", [2, 128], BF, kind="ExternalInput")
    out = nc.dram_tensor("out", [DGRP, T], F32, kind="ExternalOutput")

    with tile.TileContext(nc) as tc, ExitStack() as top:
        dram = top.enter_context(tc.tile_pool(name="dram", bufs=1, space="DRAM"))
        cc_w_in = dram.tile([1, 128], BF, name="cc_w_in")
        cc_w_out = dram.tile([4, 128], BF, name="cc_w_out")
        # qi 0-2: one combined AllGather per chunk (fewer serial CC ops);
        # qi 3: per-pair AGs so the endgame can overlap the last one
        y_loc = [dram.tile([DGRP, TQ], BF, name=f"ylq{q}") for q in range(3)]
        y_all = [dram.tile([GROUPS * DGRP, TQ], BF, name=f"yaq{q}") for q in range(3)]
        y3_loc = [dram.tile([128, TQ], BF, name=f"yl3p{p}") for p in range(1)]
        y3_all = [
            dram.tile([GROUPS * 128, TQ], BF, name=f"ya3p{p}") for p in range(1)
        ]
        # pair 1 of the last chunk gathers in two column halves so the
        # first half of the final projection can finish while the second
        # half is still in flight
        y3b_loc = [
            dram.tile([128, TQ // 2], BF, name=f"yl3b{h}") for h in range(2)
        ]
        y3b_all = [
            dram.tile([GROUPS * 128, TQ // 2], BF, name=f"ya3b{h}") for h in range(2)
        ]
        consts = top.enter_context(tc.tile_pool(name="consts", bufs=1))
        cos_sb = consts.tile([128, T], BF)
        sin_sb = consts.tile([128, T], BF)
        perm_sb = consts.tile([128, 128], BF)
        tri_sb = consts.tile([TK, TK], BF)
        bq_sb = consts.tile([128, NPAIRS], F32)
        bk_sb = consts.tile([128, NPAIRS], F32)
        bo_sb = consts.tile([128, NPAIRS], F32)
        sel_sb = consts.tile([2, 128], BF)
        bqr = bq.ap().rearrange("(p c) one -> c (p one)", c=128)
        bkr = bk.ap().rearrange("(p c) one -> c (p one)", c=128)
        bor = bo.ap().rearrange("(p c) one -> c (p one)", c=128)

        wo_pool = top.enter_context(tc.tile_pool(name="wo", bufs=1))
        wo_sb = wo_pool.tile([128, KT, DGRP], BF)

        rqk_pool = top.enter_context(tc.tile_pool(name="rqk", bufs=1))
        # RQ/RK per pair: [128, T]; rows = (u1 h0, u2 h0, u1 h1, u2 h1) x 32
        RQ = [rqk_pool.tile([128, T], BF, name=f"RQ{p}") for p in range(NPAIRS)]
        RK = [rqk_pool.tile([128, T], BF, name=f"RK{p}") for p in range(NPAIRS)]
        v_pool = top.enter_context(tc.tile_pool(name="vsb", bufs=1))
        V_sb = v_pool.tile([128, NKT, HPG, HD + 1], BF)

        # ---------------- Phase 1: QKV projection + RoPE ----------------
        with ExitStack() as ph1:
            xt_pool = ph1.enter_context(tc.tile_pool(name="xt", bufs=1))
            xT_sb = xt_pool.tile([128, NQ, KT, TQ], BF)
            wqk_pool = ph1.enter_context(tc.tile_pool(name="wqk", bufs=1))
            wq_sb = wqk_pool.tile([128, KT, DGRP], BF)
            wk_sb = wqk_pool.tile([128, KT, DGRP], BF)
            wv_sb = wqk_pool.tile([128, KT, DGRP], BF)

            # DMA issue order = completion order: feed the PE's phase-1
            # consumption sequence; wq/x-chunk-0 split per k-tile so the
            # first matmuls start as soon as ~200KB has landed
            xrv = xr.ap().rearrange("p (n k t) -> p n k t", n=NQ, k=KT)
            wqv = wqr.ap().rearrange("p (k d) -> p k d", k=KT)
            nc.gpsimd.dma_start(out=bq_sb, in_=bqr)
            nc.gpsimd.dma_start(out=bk_sb, in_=bkr)
            for h in range(2):
                ks = slice(4 * h, 4 * h + 4)
                nc.gpsimd.dma_start(out=wq_sb[:, ks, :], in_=wqv[:, ks])
                nc.gpsimd.dma_start(out=xT_sb[:, 0, ks, :], in_=xrv[:, 0, ks])
            nc.gpsimd.dma_start(out=perm_sb, in_=perm.ap())
            nc.gpsimd.dma_start(out=cos_sb[:, 0:TQ], in_=cos4.ap()[:, 0:TQ])
            nc.gpsimd.dma_start(out=sin_sb[:, 0:TQ], in_=sin4.ap()[:, 0:TQ])
            nc.gpsimd.dma_start(out=xT_sb[:, 1], in_=xrv[:, 1])
            nc.gpsimd.dma_start(
                out=cos_sb[:, TQ : 2 * TQ], in_=cos4.ap()[:, TQ : 2 * TQ]
            )
            nc.gpsimd.dma_start(
                out=sin_sb[:, TQ : 2 * TQ], in_=sin4.ap()[:, TQ : 2 * TQ]
            )
            nc.gpsimd.dma_start(out=wk_sb, in_=wkr.ap())
            nc.gpsimd.dma_start(out=xT_sb[:, 2], in_=xrv[:, 2])
            nc.gpsimd.dma_start(out=xT_sb[:, 3], in_=xrv[:, 3])
            nc.gpsimd.dma_start(out=cos_sb[:, 2 * TQ :], in_=cos4.ap()[:, 2 * TQ :])
            nc.gpsimd.dma_start(out=sin_sb[:, 2 * TQ :], in_=sin4.ap()[:, 2 * TQ :])
            nc.gpsimd.dma_start(out=wv_sb, in_=wvr.ap())
            nc.gpsimd.dma_start(out=tri_sb, in_=tri.ap())
            nc.gpsimd.dma_start(out=sel_sb, in_=sel.ap())
            nc.gpsimd.dma_start(out=bo_sb, in_=bor)
            nc.gpsimd.dma_start(out=wo_sb, in_=wor.ap())
            # warm up the CC stream so the first real AllGather doesn't pay
            # the ~11us cold-start; input is junk DRAM, nobody reads the out
            nc.gpsimd.collective_compute(
                "AllGather",
                ALU.bypass,
                ins=[cc_w_in.opt()],
                outs=[cc_w_out.opt()],
                replica_groups=[[0, 1, 2, 3], [4, 5, 6, 7]],
            )

            ps1 = ph1.enter_context(tc.tile_pool(name="ps1", bufs=2, space="PSUM"))
            tmp_pool = ph1.enter_context(tc.tile_pool(name="tmp", bufs=9))

            def rope_tail(uc, Rc, n):
                # R = u*cos + swap(u*sinp); sinp rows (+s, -s) per 64-block
                tmpU = tmp_pool.tile([128, TQ], BF, name="tmpU")
                nc.vector.tensor_mul(tmpU, uc, sin_sb[:, n * TQ : (n + 1) * TQ])
                sw_ps = ps1.tile([128, TQ], F32, name="sw_ps", bufs=3)
                nc.tensor.matmul(
                    out=sw_ps, lhsT=perm_sb, rhs=tmpU, start=True, stop=True
                )
                tmpC = tmp_pool.tile([128, TQ], F32, name="tmpC")
                nc.vector.tensor_mul(tmpC, uc, cos_sb[:, n * TQ : (n + 1) * TQ])
                nc.vector.tensor_add(Rc, tmpC, sw_ps)

            pend = None
            for p in range(NPAIRS):
                for w_sb, b_sb, R in (
                    (wq_sb, bq_sb, RQ[p]),
                    (wk_sb, bk_sb, RK[p]),
                ):
                    for n in range(NQ):
                        u_ps = ps1.tile([128, TQ], F32, name="u_ps", bufs=3)
                        for k in range(KT):
                            nc.tensor.matmul(
                                out=u_ps,
                                lhsT=w_sb[:, k, p * 128 : (p + 1) * 128],
                                rhs=xT_sb[:, n, k, :],
                                start=(k == 0),
                                stop=(k == KT - 1),
                            )
                        uc = tmp_pool.tile([128, TQ], BF, name="uc")
                        nc.scalar.activation(
                            out=uc,
                            in_=u_ps,
                            func=AF.Identity,
                            bias=b_sb[:, p : p + 1],
                        )
                        if pend is not None:
                            rope_tail(*pend)
                        pend = (uc, R[:, n * TQ : (n + 1) * TQ], n)
            rope_tail(*pend)

            # V tiles [t,d] with ones column per head
            nc.vector.memset(V_sb, 1.0)
            for tt in range(NKT):
                v_ps = ps1.tile([128, DGRP], F32, name="v_ps")
                for k in range(KT):
                    nc.tensor.matmul(
                        out=v_ps,
                        lhsT=xT_sb[:, tt // 4, k, (tt % 4) * TK : (tt % 4 + 1) * TK],
                        rhs=wv_sb[:, k, :],
                        start=(k == 0),
                        stop=(k == KT - 1),
                    )
                nc.vector.tensor_copy(
                    out=V_sb[:, tt, :, 0:HD],
                    in_=v_ps.rearrange("p (h d) -> p h d", h=HPG),
                )

        # ---------- Phase 2+3: attention, chunked AG + projection ----------
        with ExitStack() as ph2:
            sd_pool = ph2.enter_context(tc.tile_pool(name="sduo", bufs=2, space="PSUM"))
            av_pool = ph2.enter_context(tc.tile_pool(name="av", bufs=1, space="PSUM"))
            bc_pool = ph2.enter_context(tc.tile_pool(name="bc", bufs=1, space="PSUM"))
            ps3 = ph2.enter_context(tc.tile_pool(name="ps3", bufs=1, space="PSUM"))
            pt_pool = ph2.enter_context(tc.tile_pool(name="ptile", bufs=3))
            yt_pool = ph2.enter_context(tc.tile_pool(name="yt", bufs=2))
            ya_pool = ph2.enter_context(tc.tile_pool(name="ya", bufs=2))
            ob_pool = ph2.enter_context(tc.tile_pool(name="ob", bufs=2))
            oa_pool = ph2.enter_context(tc.tile_pool(name="oa", bufs=2))
            rb_pool = ph2.enter_context(tc.tile_pool(name="rb", bufs=4))
            av = [av_pool.tile([128, TQ], F32, name=f"av{hh}") for hh in range(2)]

            def issue_av(p, qi, ptile, g2, hh):
                last_ti = 4 * qi + 3
                for ji in range(2):
                    ti = 2 * g2 + ji
                    nc.tensor.matmul(
                        out=av[hh][0:65, :],
                        lhsT=V_sb[:, ti, 2 * p + hh, :],
                        rhs=ptile[:, ji * TQ : (ji + 1) * TQ],
                        start=(ti == 0),
                        stop=(ti == last_ti),
                    )

            def proj_dma(qi, ks):
                ya_sb = ya_pool.tile([128, KT, TQ], BF, name="ya_sb")
                for k in ks:
                    if qi < 3:
                        srcv = y_all[qi][k * 128 : (k + 1) * 128, :]
                    else:
                        srcv = y3_all[k % 2][(k // 2) * 128 : (k // 2 + 1) * 128, :]
                    nc.gpsimd.dma_start(out=ya_sb[:, k, :], in_=srcv)
                return ya_sb

            def proj_chunk(qi, korder=tuple(range(KT)), ya_sb=None):
                if ya_sb is None:
                    ya_sb = proj_dma(qi, korder)
                for mb in range(2):
                    o_ps = ps3.tile([128, TQ], F32, name="o_ps")
                    for i, k in enumerate(korder):
                        nc.tensor.matmul(
                            out=o_ps,
                            lhsT=wo_sb[:, k, mb * 128 : (mb + 1) * 128],
                            rhs=ya_sb[:, k, :],
                            start=(i == 0),
                            stop=(i == KT - 1),
                        )
                    ob = ob_pool.tile([128, TQ], F32, name="ob")
                    nc.vector.tensor_scalar_add(
                        out=ob, in0=o_ps, scalar1=bo_sb[:, mb : mb + 1]
                    )
                    nc.gpsimd.dma_start(
                        out=out.ap()[
                            mb * 128 : (mb + 1) * 128, qi * TQ : (qi + 1) * TQ
                        ],
                        in_=ob,
                    )

            for qi in range(NQ):
                if qi >= 2:
                    # fill the AG-latency window of chunk qi-2 with its proj
                    proj_chunk(qi - 2)
                for p in range(NPAIRS):
                    avpend = None
                    for g2 in range(2 * qi + 2):
                        diag0 = g2 == 2 * qi
                        diag1 = g2 == 2 * qi + 1
                        for hh in range(2):
                            s_ps = sd_pool.tile([128, 2 * TQ], F32, name="s_ps")
                            for ji in range(2):
                                ti = 2 * g2 + ji
                                c0 = TK * (ti - 4 * qi) if diag1 else 0
                                nc.tensor.matmul(
                                    out=s_ps[:, ji * TQ + c0 : (ji + 1) * TQ],
                                    lhsT=RK[p][
                                        64 * hh : 64 * hh + 64,
                                        ti * TK : (ti + 1) * TK,
                                    ],
                                    rhs=RQ[p][
                                        64 * hh : 64 * hh + 64,
                                        qi * TQ + c0 : (qi + 1) * TQ,
                                    ],
                                    start=True,
                                    stop=True,
                                )
                            ptile = pt_pool.tile([128, 2 * TQ], BF, name="ptile")
                            if diag1:
                                # only [256:512] and [896:1024] computed
                                nc.scalar.activation(
                                    out=ptile[:, 256:512],
                                    in_=s_ps[:, 256:512],
                                    func=AF.Exp,
                                    scale=SCALE,
                                )
                                nc.scalar.activation(
                                    out=ptile[:, 896:1024],
                                    in_=s_ps[:, 896:1024],
                                    func=AF.Exp,
                                    scale=SCALE,
                                )
                                nc.vector.memset(ptile[:, 0:256], 0.0)
                                nc.vector.memset(ptile[:, 512:896], 0.0)
                                nc.vector.tensor_mul(
                                    ptile[:, 256:384], ptile[:, 256:384], tri_sb
                                )
                                nc.vector.tensor_mul(
                                    ptile[:, 896:1024], ptile[:, 896:1024], tri_sb
                                )
                            else:
                                nc.scalar.activation(
                                    out=ptile, in_=s_ps, func=AF.Exp, scale=SCALE
                                )
                                if diag0:
                                    nc.vector.memset(ptile[:, 512:640], 0.0)
                                    nc.vector.tensor_mul(
                                        ptile[:, 0:128], ptile[:, 0:128], tri_sb
                                    )
                                    nc.vector.tensor_mul(
                                        ptile[:, 640:768], ptile[:, 640:768], tri_sb
                                    )
                            if avpend is not None:
                                issue_av(p, qi, *avpend)
                            avpend = (ptile, g2, hh)
                    issue_av(p, qi, *avpend)

                    # evict O_aug to SBUF immediately so the shared av PSUM
                    # tiles free fast (next pair's AV matmuls reuse them);
                    # the whole normalize chain then runs off-critical-path.
                    # Denom rows stage in free dim of one partition (engine
                    # partition bases must be 32-aligned), DMA-scatter to 2
                    # partitions, approx-reciprocal on DVE (ACT keeps its
                    # Exp table loaded), broadcast via PE
                    oa = oa_pool.tile([128, TQ], F32, name="oa")
                    dq = rb_pool.tile([1, 2, TQ], F32, name="dq")
                    nc.vector.tensor_copy(out=oa[0:64, :], in_=av[0][0:64, :])
                    nc.vector.tensor_copy(out=oa[64:128, :], in_=av[1][0:64, :])
                    nc.vector.tensor_copy(out=dq[0:1, 0, :], in_=av[0][64:65, :])
                    nc.vector.tensor_copy(out=dq[0:1, 1, :], in_=av[1][64:65, :])
                    rbuf = rb_pool.tile([2, TQ], F32, name="rbuf")
                    nc.gpsimd.dma_start(out=rbuf, in_=dq[0:1, :, :])
                    rinv = rb_pool.tile([2, TQ], F32, name="rinv")
                    nc.vector.reciprocal_approx_fast(rinv, rbuf)
                    rcast = rb_pool.tile([2, TQ], BF, name="rcast")
                    nc.vector.tensor_copy(out=rcast, in_=rinv)
                    bc_ps = bc_pool.tile([128, TQ], F32, name="bc_ps")
                    nc.tensor.matmul(
                        out=bc_ps,
                        lhsT=sel_sb,
                        rhs=rcast,
                        start=True,
                        stop=True,
                    )
                    yt2 = yt_pool.tile([128, TQ], BF, name="yt2")
                    nc.vector.tensor_mul(yt2[0:64, :], oa[0:64, :], bc_ps[0:64, :])
                    nc.vector.tensor_mul(
                        yt2[64:128, :], oa[64:128, :], bc_ps[64:128, :]
                    )
                    if qi < 3:
                        nc.gpsimd.dma_start(
                            out=y_loc[qi][p * 128 : (p + 1) * 128, :], in_=yt2
                        )
                        if p == 1:
                            nc.gpsimd.collective_compute(
                                "AllGather",
                                ALU.bypass,
                                ins=[y_loc[qi].opt()],
                                outs=[y_all[qi].opt()],
                                replica_groups=[[0, 1, 2, 3], [4, 5, 6, 7]],
                            )
                    elif p == 0:
                        nc.gpsimd.dma_start(out=y3_loc[0][:, :], in_=yt2)
                        nc.gpsimd.collective_compute(
                            "AllGather",
                            ALU.bypass,
                            ins=[y3_loc[0].opt()],
                            outs=[y3_all[0].opt()],
                            replica_groups=[[0, 1, 2, 3], [4, 5, 6, 7]],
                        )
                    else:
                        for h in range(2):
                            nc.gpsimd.dma_start(
                                out=y3b_loc[h][:, :],
                                in_=yt2[:, h * 256 : (h + 1) * 256],
                            )
                            nc.gpsimd.collective_compute(
                                "AllGather",
                                ALU.bypass,
                                ins=[y3b_loc[h].opt()],
                                outs=[y3b_all[h].opt()],
                                replica_groups=[[0, 1, 2, 3], [4, 5, 6, 7]],
                            )
                    if qi == NQ - 1 and p == 0:
                        # start chunk-2 ya transfers early (clears the DMA
                        # ring before the endgame); MMs run after p1's work
                        ya2 = proj_dma(2, range(KT))
            proj_chunk(2, ya_sb=ya2)
            # --- endgame for chunk 3: even k-tiles were gathered by
            # AG(3,p0) long ago - load + matmul them while AG(3,p1) runs;
            # mb1 borrows the bc PSUM bank (free after the last epilogue)
            evens, odds = (0, 2, 4, 6), (1, 3, 5, 7)
            ya3 = proj_dma(3, evens)
            o30 = ps3.tile([128, TQ], F32, name="o_ps")
            for i, k in enumerate(evens):
                nc.tensor.matmul(
                    out=o30, lhsT=wo_sb[:, k, 0:128], rhs=ya3[:, k, :],
                    start=(i == 0), stop=False,
                )
            o31 = bc_pool.tile([128, TQ], F32, name="bc_ps")
            for i, k in enumerate(evens):
                nc.tensor.matmul(
                    out=o31, lhsT=wo_sb[:, k, 128:256], rhs=ya3[:, k, :],
                    start=(i == 0), stop=False,
                )
            for h in range(2):
                hs = slice(h * 256, (h + 1) * 256)
                for k in odds:
                    nc.gpsimd.dma_start(
                        out=ya3[:, k, hs],
                        in_=y3b_all[h][(k // 2) * 128 : (k // 2 + 1) * 128, :],
                    )
                for i, k in enumerate(odds):
                    nc.tensor.matmul(
                        out=o30[:, hs], lhsT=wo_sb[:, k, 0:128],
                        rhs=ya3[:, k, hs], start=False, stop=(i == 3),
                    )
                    nc.tensor.matmul(
                        out=o31[:, hs], lhsT=wo_sb[:, k, 128:256],
                        rhs=ya3[:, k, hs], start=False, stop=(i == 3),
                    )
                for mb, o_ps in ((0, o30), (1, o31)):
                    ob = ob_pool.tile([128, TQ // 2], F32, name="ob")
                    nc.vector.tensor_scalar_add(
                        out=ob, in0=o_ps[:, hs], scalar1=bo_sb[:, mb : mb + 1]
                    )
                    nc.gpsimd.dma_start(
                        out=out.ap()[
                            mb * 128 : (mb + 1) * 128,
                            3 * TQ + h * 256 : 3 * TQ + (h + 1) * 256,
                        ],
                        in_=ob,
                    )
    nc.finalize()
    return nc


_NC = None


def _get_nc():
    global _NC
    if _NC is None:
        _NC = build_nc()
    return _NC


def _relay(w):
    # [KT*128, M] -> [128, KT*M] so each partition's DMA line is contiguous
    kt, m = w.shape[0] // 128, w.shape[1]
    return np.ascontiguousarray(
        w.reshape(kt, 128, m).transpose(1, 0, 2).reshape(128, kt * m).astype(BF_NP)
    )


def _relay_x(xb):
    # x^T [C, T] -> [128, NQ*KT*TQ] n-major so each 512-col block is one
    # contiguous-per-partition DMA
    xt = xb.T.reshape(KT, 128, NQ, TQ)
    return np.ascontiguousarray(
        xt.transpose(1, 2, 0, 3).reshape(128, NQ * KT * TQ).astype(BF_NP)
    )


def _in_maps(x, freqs_cos, freqs_sin, Wqkv, bqkv, Wproj, bproj):
    x = np.asarray(x, np.float32)
    Wqkv = np.asarray(Wqkv, np.float32)
    bqkv = np.asarray(bqkv, np.float32)
    Wproj = np.asarray(Wproj, np.float32)
    bproj = np.asarray(bproj, np.float32)
    cosT = np.asarray(freqs_cos, np.float32).T  # [32, T]
    sinT = np.asarray(freqs_sin, np.float32).T
    cos4 = np.ascontiguousarray(np.tile(cosT, (4, 1)).astype(BF_NP))
    # sinp rows (+s, -s) per 64-block: R = u*cos + swap(u*sinp)
    sin4 = np.ascontiguousarray(
        np.tile(np.concatenate([sinT, -sinT], axis=0), (2, 1)).astype(BF_NP)
    )
    perm = np.zeros((128, 128), np.float32)
    for j in range(128):
        i = j + 32 if (j % 64) < 32 else j - 32
        perm[i, j] = 1.0
    perm = perm.astype(BF_NP)
    tri = np.triu(np.ones((TK, TK), np.float32)).astype(BF_NP)
    sel = np.zeros((2, 128), np.float32)
    sel[0, 0:64] = 1.0
    sel[1, 64:128] = 1.0
    sel = sel.astype(BF_NP)
    bproj_eff = bproj + bqkv[2 * C : 3 * C] @ Wproj
    maps = []
    for r in range(NCORES):
        b, g = r // GROUPS, r % GROUPS
        sl = slice(DGRP * g, DGRP * (g + 1))
        maps.append(
            {
                "xr": _relay_x(x[b]),
                "wqr": _relay(Wqkv[:, 0 * C :][:, sl]),
                "wkr": _relay(Wqkv[:, 1 * C :][:, sl]),
                "wvr": _relay(Wqkv[:, 2 * C :][:, sl]),
                "wor": _relay(Wproj[:, sl]),
                "cos4": cos4,
                "sin4": sin4,
                "perm": perm,
                "tri": tri,
                "sel": sel,
                "bq": np.ascontiguousarray(bqkv[0 * C : 1 * C][sl]).reshape(DGRP, 1),
                "bk": np.ascontiguousarray(bqkv[1 * C : 2 * C][sl]).reshape(DGRP, 1),
                "bo": np.ascontiguousarray(bproj_eff[sl]).reshape(DGRP, 1),
            }
        )
    return maps


def _assemble(results):
    y = np.empty((B, T, C), np.float32)
    for b in range(B):
        cat = np.concatenate(
            [np.asarray(results[GROUPS * b + g]["out"]) for g in range(GROUPS)], axis=0
        )
        y[b] = cat.T
    return y


def kernel(**inputs):
    nc = _get_nc()
    res = run_bass_kernel_spmd(nc, _in_maps(**inputs), core_ids=list(range(NCORES)))
    return _assemble(res.results)


def kernel_traced(**inputs):
    import tempfile

    nc = _get_nc()
    tmpdir = tempfile.mkdtemp(prefix="mha_trace_")
    res = run_bass_kernel_spmd(
        nc,
        _in_maps(**inputs),
        core_ids=list(range(NCORES)),
        trace=True,
        trace_cores=list(range(NCORES)),
        tmpdir=tmpdir,
    )
    return _assemble(res.results), res.exec_time_ns, tmpdir


# revision 18
# speedup vs baseline: 1.0257x; 1.0257x over previous
"""MHA kernel for trn2: 8 cores = 2 (batch DP) x 4 (head TP, 4 heads/core).

Layout strategy (all device tensors d-major / transposed so no on-device
transposes are ever needed); all matmul operands in bf16 (PE runs 1
cycle/row vs 2 for fp32 HIGH mode), accumulation stays fp32 in PSUM:
  - x^T [C, T] per batch (host-transposed, partition-relayout, bf16)
  - Q^T/K^T computed as [d, t] via lhsT=W-slice, rhs=x^T; RoPE via
    multiply-by-signed-sin then PE half-swap permutation matmul (all DVE
    ops same-dtype: bf16*bf16 or f32+f32)
  - V computed as [t, d] bf16 (plus ones column for softmax denominators)
  - S^T duos [tk=128, 2*512]; 2-head packing (K=64, base partitions
    0/64); causal block skipping + diagonal S/exp trimming; exp (bf16
    out) per duo; software-pipelined AV one duo behind S
  - AV accumulates O_aug^T [65, tq] per head; row 64 = softmax denom
  - denominators DMA'd from PSUM rows, reciprocal on DVE (keeps ACT on
    the Exp table only - no table thrash), PE broadcast matmul, per-pair
    normalize directly from the AV PSUM tiles
  - AllGather y^T per (chunk, pair) [128, 512] bf16 (8 small CC ops)
    overlapped with the next pair/chunk; projection per chunk one step
    behind; the final chunk's projection reads even k-tiles (gathered
    first) before odd ones to shorten the serial tail
Host reassembles: concat cout slices, transpose, stack batches.
"""

import sys

sys.path.insert(0, "/opt/trn_rl_repo")

from contextlib import ExitStack  # noqa: E402

import numpy as np  # noqa: E402
import ml_dtypes  # noqa: E402

import concourse.bacc as bacc  # noqa: E402
import concourse.bass as bass  # noqa: E402
import concourse.tile as tile  # noqa: E402
from concourse import mybir  # noqa: E402
from concourse.bass_utils import run_bass_kernel_spmd  # noqa: E402

B, T, C, H = 2, 2048, 1024, 16
HD, HD2 = 64, 32
NCORES, GROUPS, HPG, NPAIRS = 8, 4, 4, 2
TK, TQ = 128, 512
NQ = T // TQ  # 4 q-chunks
NKT = T // TK  # 16 tk tiles
KT = C // 128  # 8 contraction tiles
DGRP = 256  # head dims per core (4 heads * 64)

BF = mybir.dt.bfloat16
F32 = mybir.dt.float32
F32R = mybir.dt.float32r
AF = mybir.ActivationFunctionType
ALU = mybir.AluOpType
SCALE = 1.0 / 8.0  # 1/sqrt(HD)
BF_NP = ml_dtypes.bfloat16


def r32(ap):
    return ap.bitcast(F32R)


def build_nc():
    nc = bacc.Bacc(target_bir_lowering=False)

    xr = nc.dram_tensor("xr", [128, NQ * KT * TQ], BF, kind="ExternalInput")
    wqr = nc.dram_tensor("wqr", [128, KT * DGRP], BF, kind="ExternalInput")
    wkr = nc.dram_tensor("wkr", [128, KT * DGRP], BF, kind="ExternalInput")
    wvr = nc.dram_tensor("wvr", [128, KT * DGRP], BF, kind="ExternalInput")
    wor = nc.dram_tensor("wor", [128, KT * DGRP], BF, kind="ExternalInput")
    cos4 = nc.dram_tensor("cos4", [128, T], BF, kind="ExternalInput")
    sin4 = nc.dram_tensor("sin4", [128, T], BF, kind="ExternalInput")
    perm = nc.dram_tensor("perm", [128, 128], BF, kind="ExternalInput")
    tri = nc.dram_tensor("tri", [TK, TK], BF, kind="ExternalInput")
    bq = nc.dram_tensor("bq", [DGRP, 1], F32, kind="ExternalInput")
    bk = nc.dram_tensor("bk", [DGRP, 1], F32, kind="ExternalInput")
    bo = nc.dram_tensor("bo", [DGRP, 1], F32, kind="ExternalInput")
    sel = nc.dram_tensor("sel", [2, 128], BF, kind="ExternalInput")
    out = nc.dram_tensor("out", [DGRP, T], F32, kind="ExternalOutput")

    with tile.TileContext(nc) as tc, ExitStack() as top:
        dram = top.enter_context(tc.tile_pool(name="dram", bufs=1, space="DRAM"))
        cc_w_in = dram.tile([1, 128], BF, name="cc_w_in")
        cc_w_out = dram.tile([4, 128], BF, name="cc_w_out")
        # qi 0-2: one combined AllGather per chunk (fewer serial CC ops);
        # qi 3: per-pair AGs so the endgame can overlap the last one
        y_loc = [dram.tile([DGRP, TQ], BF, name=f"ylq{q}") for q in range(3)]
        y_all = [dram.tile([GROUPS * DGRP, TQ], BF, name=f"yaq{q}") for q in range(3)]
        y3_loc = [dram.tile([128, TQ], BF, name=f"yl3p{p}") for p in range(NPAIRS)]
        y3_all = [
            dram.tile([GROUPS * 128, TQ], BF, name=f"ya3p{p}") for p in range(NPAIRS)
        ]
        consts = top.enter_context(tc.tile_pool(name="consts", bufs=1))
        cos_sb = consts.tile([128, T], BF)
        sin_sb = consts.tile([128, T], BF)
        perm_sb = consts.tile([128, 128], BF)
        tri_sb = consts.tile([TK, TK], BF)
        bq_sb = consts.tile([128, NPAIRS], F32)
        bk_sb = consts.tile([128, NPAIRS], F32)
        bo_sb = consts.tile([128, NPAIRS], F32)
        sel_sb = consts.tile([2, 128], BF)
        bqr = bq.ap().rearrange("(p c) one -> c (p one)", c=128)
        bkr = bk.ap().rearrange("(p c) one -> c (p one)", c=128)
        bor = bo.ap().rearrange("(p c) one -> c (p one)", c=128)

        wo_pool = top.enter_context(tc.tile_pool(name="wo", bufs=1))
        wo_sb = wo_pool.tile([128, KT, DGRP], BF)

        rqk_pool = top.enter_context(tc.tile_pool(name="rqk", bufs=1))
        # RQ/RK per pair: [128, T]; rows = (u1 h0, u2 h0, u1 h1, u2 h1) x 32
        RQ = [rqk_pool.tile([128, T], BF, name=f"RQ{p}") for p in range(NPAIRS)]
        RK = [rqk_pool.tile([128, T], BF, name=f"RK{p}") for p in range(NPAIRS)]
        v_pool = top.enter_context(tc.tile_pool(name="vsb", bufs=1))
        V_sb = v_pool.tile([128, NKT, HPG, HD + 1], BF)

        # ---------------- Phase 1: QKV projection + RoPE ----------------
        with ExitStack() as ph1:
            xt_pool = ph1.enter_context(tc.tile_pool(name="xt", bufs=1))
            xT_sb = xt_pool.tile([128, NQ, KT, TQ], BF)
            wqk_pool = ph1.enter_context(tc.tile_pool(name="wqk", bufs=1))
            wq_sb = wqk_pool.tile([128, KT, DGRP], BF)
            wk_sb = wqk_pool.tile([128, KT, DGRP], BF)
            wv_sb = wqk_pool.tile([128, KT, DGRP], BF)

            # DMA issue order = completion order: feed the PE's phase-1
            # consumption sequence; wq/x-chunk-0 split per k-tile so the
            # first matmuls start as soon as ~200KB has landed
            xrv = xr.ap().rearrange("p (n k t) -> p n k t", n=NQ, k=KT)
            wqv = wqr.ap().rearrange("p (k d) -> p k d", k=KT)
            nc.gpsimd.dma_start(out=bq_sb, in_=bqr)
            nc.gpsimd.dma_start(out=bk_sb, in_=bkr)
            for h in range(2):
                ks = slice(4 * h, 4 * h + 4)
                nc.gpsimd.dma_start(out=wq_sb[:, ks, :], in_=wqv[:, ks])
                nc.gpsimd.dma_start(out=xT_sb[:, 0, ks, :], in_=xrv[:, 0, ks])
            nc.gpsimd.dma_start(out=perm_sb, in_=perm.ap())
            nc.gpsimd.dma_start(out=cos_sb[:, 0:TQ], in_=cos4.ap()[:, 0:TQ])
            nc.gpsimd.dma_start(out=sin_sb[:, 0:TQ], in_=sin4.ap()[:, 0:TQ])
            nc.gpsimd.dma_start(out=xT_sb[:, 1], in_=xrv[:, 1])
            nc.gpsimd.dma_start(
                out=cos_sb[:, TQ : 2 * TQ], in_=cos4.ap()[:, TQ : 2 * TQ]
            )
            nc.gpsimd.dma_start(
                out=sin_sb[:, TQ : 2 * TQ], in_=sin4.ap()[:, TQ : 2 * TQ]
            )
            nc.gpsimd.dma_start(out=wk_sb, in_=wkr.ap())
            nc.gpsimd.dma_start(out=xT_sb[:, 2], in_=xrv[:, 2])
            nc.gpsimd.dma_start(out=xT_sb[:, 3], in_=xrv[:, 3])
            nc.gpsimd.dma_start(out=cos_sb[:, 2 * TQ :], in_=cos4.ap()[:, 2 * TQ :])
            nc.gpsimd.dma_start(out=sin_sb[:, 2 * TQ :], in_=sin4.ap()[:, 2 * TQ :])
            nc.gpsimd.dma_start(out=wv_sb, in_=wvr.ap())
            nc.gpsimd.dma_start(out=tri_sb, in_=tri.ap())
            nc.gpsimd.dma_start(out=sel_sb, in_=sel.ap())
            nc.gpsimd.dma_start(out=bo_sb, in_=bor)
            nc.gpsimd.dma_start(out=wo_sb, in_=wor.ap())
            # warm up the CC stream so the first real AllGather doesn't pay
            # the ~11us cold-start; input is junk DRAM, nobody reads the out
            nc.gpsimd.collective_compute(
                "AllGather",
                ALU.bypass,
                ins=[cc_w_in.opt()],
                outs=[cc_w_out.opt()],
                replica_groups=[[0, 1, 2, 3], [4, 5, 6, 7]],
            )

            ps1 = ph1.enter_context(tc.tile_pool(name="ps1", bufs=2, space="PSUM"))
            tmp_pool = ph1.enter_context(tc.tile_pool(name="tmp", bufs=9))

            def rope_tail(uc, Rc, n):
                # R = u*cos + swap(u*sinp); sinp rows (+s, -s) per 64-block
                tmpU = tmp_pool.tile([128, TQ], BF, name="tmpU")
                nc.vector.tensor_mul(tmpU, uc, sin_sb[:, n * TQ : (n + 1) * TQ])
                sw_ps = ps1.tile([128, TQ], F32, name="sw_ps", bufs=3)
                nc.tensor.matmul(
                    out=sw_ps, lhsT=perm_sb, rhs=tmpU, start=True, stop=True
                )
                tmpC = tmp_pool.tile([128, TQ], F32, name="tmpC")
                nc.vector.tensor_mul(tmpC, uc, cos_sb[:, n * TQ : (n + 1) * TQ])
                nc.vector.tensor_add(Rc, tmpC, sw_ps)

            pend = None
            for p in range(NPAIRS):
                for w_sb, b_sb, R in (
                    (wq_sb, bq_sb, RQ[p]),
                    (wk_sb, bk_sb, RK[p]),
                ):
                    for n in range(NQ):
                        u_ps = ps1.tile([128, TQ], F32, name="u_ps", bufs=3)
                        for k in range(KT):
                            nc.tensor.matmul(
                                out=u_ps,
                                lhsT=w_sb[:, k, p * 128 : (p + 1) * 128],
                                rhs=xT_sb[:, n, k, :],
                                start=(k == 0),
                                stop=(k == KT - 1),
                            )
                        uc = tmp_pool.tile([128, TQ], BF, name="uc")
                        nc.scalar.activation(
                            out=uc,
                            in_=u_ps,
                            func=AF.Identity,
                            bias=b_sb[:, p : p + 1],
                        )
                        if pend is not None:
                            rope_tail(*pend)
                        pend = (uc, R[:, n * TQ : (n + 1) * TQ], n)
            rope_tail(*pend)

            # V tiles [t,d] with ones column per head
            nc.vector.memset(V_sb, 1.0)
            for tt in range(NKT):
                v_ps = ps1.tile([128, DGRP], F32, name="v_ps")
                for k in range(KT):
                    nc.tensor.matmul(
                        out=v_ps,
                        lhsT=xT_sb[:, tt // 4, k, (tt % 4) * TK : (tt % 4 + 1) * TK],
                        rhs=wv_sb[:, k, :],
                        start=(k == 0),
                        stop=(k == KT - 1),
                    )
                nc.vector.tensor_copy(
                    out=V_sb[:, tt, :, 0:HD],
                    in_=v_ps.rearrange("p (h d) -> p h d", h=HPG),
                )

        # ---------- Phase 2+3: attention, chunked AG + projection ----------
        with ExitStack() as ph2:
            sd_pool = ph2.enter_context(tc.tile_pool(name="sduo", bufs=2, space="PSUM"))
            av_pool = ph2.enter_context(tc.tile_pool(name="av", bufs=1, space="PSUM"))
            bc_pool = ph2.enter_context(tc.tile_pool(name="bc", bufs=1, space="PSUM"))
            ps3 = ph2.enter_context(tc.tile_pool(name="ps3", bufs=1, space="PSUM"))
            pt_pool = ph2.enter_context(tc.tile_pool(name="ptile", bufs=3))
            yt_pool = ph2.enter_context(tc.tile_pool(name="yt", bufs=2))
            ya_pool = ph2.enter_context(tc.tile_pool(name="ya", bufs=2))
            ob_pool = ph2.enter_context(tc.tile_pool(name="ob", bufs=2))
            oa_pool = ph2.enter_context(tc.tile_pool(name="oa", bufs=2))
            rb_pool = ph2.enter_context(tc.tile_pool(name="rb", bufs=4))
            av = [av_pool.tile([128, TQ], F32, name=f"av{hh}") for hh in range(2)]

            def issue_av(p, qi, ptile, g2, hh):
                last_ti = 4 * qi + 3
                for ji in range(2):
                    ti = 2 * g2 + ji
                    nc.tensor.matmul(
                        out=av[hh][0:65, :],
                        lhsT=V_sb[:, ti, 2 * p + hh, :],
                        rhs=ptile[:, ji * TQ : (ji + 1) * TQ],
                        start=(ti == 0),
                        stop=(ti == last_ti),
                    )

            def proj_dma(qi, ks):
                ya_sb = ya_pool.tile([128, KT, TQ], BF, name="ya_sb")
                for k in ks:
                    if qi < 3:
                        srcv = y_all[qi][k * 128 : (k + 1) * 128, :]
                    else:
                        srcv = y3_all[k % 2][(k // 2) * 128 : (k // 2 + 1) * 128, :]
                    nc.gpsimd.dma_start(out=ya_sb[:, k, :], in_=srcv)
                return ya_sb

            def proj_chunk(qi, korder=tuple(range(KT)), ya_sb=None):
                if ya_sb is None:
                    ya_sb = proj_dma(qi, korder)
                for mb in range(2):
                    o_ps = ps3.tile([128, TQ], F32, name="o_ps")
                    for i, k in enumerate(korder):
                        nc.tensor.matmul(
                            out=o_ps,
                            lhsT=wo_sb[:, k, mb * 128 : (mb + 1) * 128],
                            rhs=ya_sb[:, k, :],
                            start=(i == 0),
                            stop=(i == KT - 1),
                        )
                    ob = ob_pool.tile([128, TQ], F32, name="ob")
                    nc.vector.tensor_scalar_add(
                        out=ob, in0=o_ps, scalar1=bo_sb[:, mb : mb + 1]
                    )
                    nc.gpsimd.dma_start(
                        out=out.ap()[
                            mb * 128 : (mb + 1) * 128, qi * TQ : (qi + 1) * TQ
                        ],
                        in_=ob,
                    )

            for qi in range(NQ):
                if qi >= 2:
                    # fill the AG-latency window of chunk qi-2 with its proj
                    proj_chunk(qi - 2)
                for p in range(NPAIRS):
                    avpend = None
                    for g2 in range(2 * qi + 2):
                        diag0 = g2 == 2 * qi
                        diag1 = g2 == 2 * qi + 1
                        for hh in range(2):
                            s_ps = sd_pool.tile([128, 2 * TQ], F32, name="s_ps")
                            for ji in range(2):
                                ti = 2 * g2 + ji
                                c0 = TK * (ti - 4 * qi) if diag1 else 0
                                nc.tensor.matmul(
                                    out=s_ps[:, ji * TQ + c0 : (ji + 1) * TQ],
                                    lhsT=RK[p][
                                        64 * hh : 64 * hh + 64,
                                        ti * TK : (ti + 1) * TK,
                                    ],
                                    rhs=RQ[p][
                                        64 * hh : 64 * hh + 64,
                                        qi * TQ + c0 : (qi + 1) * TQ,
                                    ],
                                    start=True,
                                    stop=True,
                                )
                            ptile = pt_pool.tile([128, 2 * TQ], BF, name="ptile")
                            if diag1:
                                # only [256:512] and [896:1024] computed
                                nc.scalar.activation(
                                    out=ptile[:, 256:512],
                                    in_=s_ps[:, 256:512],
                                    func=AF.Exp,
                                    scale=SCALE,
                                )
                                nc.scalar.activation(
                                    out=ptile[:, 896:1024],
                                    in_=s_ps[:, 896:1024],
                                    func=AF.Exp,
                                    scale=SCALE,
                                )
                                nc.vector.memset(ptile[:, 0:256], 0.0)
                                nc.vector.memset(ptile[:, 512:896], 0.0)
                                nc.vector.tensor_mul(
                                    ptile[:, 256:384], ptile[:, 256:384], tri_sb
                                )
                                nc.vector.tensor_mul(
                                    ptile[:, 896:1024], ptile[:, 896:1024], tri_sb
                                )
                            else:
                                nc.scalar.activation(
                                    out=ptile, in_=s_ps, func=AF.Exp, scale=SCALE
                                )
                                if diag0:
                                    nc.vector.memset(ptile[:, 512:640], 0.0)
                                    nc.vector.tensor_mul(
                                        ptile[:, 0:128], ptile[:, 0:128], tri_sb
                                    )
                                    nc.vector.tensor_mul(
                                        ptile[:, 640:768], ptile[:, 640:768], tri_sb
                                    )
                            if avpend is not None:
                                issue_av(p, qi, *avpend)
                            avpend = (ptile, g2, hh)
                    issue_av(p, qi, *avpend)

                    # evict O_aug to SBUF immediately so the shared av PSUM
                    # tiles free fast (next pair's AV matmuls reuse them);
                    # the whole normalize chain then runs off-critical-path.
                    # Denom rows stage in free dim of one partition (engine
                    # partition bases must be 32-aligned), DMA-scatter to 2
                    # partitions, approx-reciprocal on DVE (ACT keeps its
                    # Exp table loaded), broadcast via PE
                    oa = oa_pool.tile([128, TQ], F32, name="oa")
                    dq = rb_pool.tile([1, 2, TQ], F32, name="dq")
                    nc.vector.tensor_copy(out=oa[0:64, :], in_=av[0][0:64, :])
                    nc.vector.tensor_copy(out=oa[64:128, :], in_=av[1][0:64, :])
                    nc.vector.tensor_copy(out=dq[0:1, 0, :], in_=av[0][64:65, :])
                    nc.vector.tensor_copy(out=dq[0:1, 1, :], in_=av[1][64:65, :])
                    rbuf = rb_pool.tile([2, TQ], F32, name="rbuf")
                    nc.gpsimd.dma_start(out=rbuf, in_=dq[0:1, :, :])
                    rinv = rb_pool.tile([2, TQ], F32, name="rinv")
                    nc.vector.reciprocal_approx_fast(rinv, rbuf)
                    rcast = rb_pool.tile([2, TQ], BF, name="rcast")
                    nc.vector.tensor_copy(out=rcast, in_=rinv)
                    bc_ps = bc_pool.tile([128, TQ], F32, name="bc_ps")
                    nc.tensor.matmul(
                        out=bc_ps,
                        lhsT=sel_sb,
                        rhs=rcast,
                        start=True,
                        stop=True,
                    )
                    yt2 = yt_pool.tile([128, TQ], BF, name="yt2")
                    nc.vector.tensor_mul(yt2[0:64, :], oa[0:64, :], bc_ps[0:64, :])
                    nc.vector.tensor_mul(
                        yt2[64:128, :], oa[64:128, :], bc_ps[64:128, :]
                    )
                    if qi < 3:
                        nc.gpsimd.dma_start(
                            out=y_loc[qi][p * 128 : (p + 1) * 128, :], in_=yt2
                        )
                        if p == 1:
                            nc.gpsimd.collective_compute(
                                "AllGather",
                                ALU.bypass,
                                ins=[y_loc[qi].opt()],
                                outs=[y_all[qi].opt()],
                                replica_groups=[[0, 1, 2, 3], [4, 5, 6, 7]],
                            )
                    else:
                        nc.gpsimd.dma_start(out=y3_loc[p][:, :], in_=yt2)
                        nc.gpsimd.collective_compute(
                            "AllGather",
                            ALU.bypass,
                            ins=[y3_loc[p].opt()],
                            outs=[y3_all[p].opt()],
                            replica_groups=[[0, 1, 2, 3], [4, 5, 6, 7]],
                        )
                    if qi == NQ - 1 and p == 0:
                        # start chunk-2 ya transfers early (clears the DMA
                        # ring before the endgame); MMs run after p1's work
                        ya2 = proj_dma(2, range(KT))
            proj_chunk(2, ya_sb=ya2)
            # --- endgame for chunk 3: even k-tiles were gathered by
            # AG(3,p0) long ago - load + matmul them while AG(3,p1) runs;
            # mb1 borrows the bc PSUM bank (free after the last epilogue)
            evens, odds = (0, 2, 4, 6), (1, 3, 5, 7)
            ya3 = proj_dma(3, evens)
            o30 = ps3.tile([128, TQ], F32, name="o_ps")
            for i, k in enumerate(evens):
                nc.tensor.matmul(
                    out=o30, lhsT=wo_sb[:, k, 0:128], rhs=ya3[:, k, :],
                    start=(i == 0), stop=False,
                )
            o31 = bc_pool.tile([128, TQ], F32, name="bc_ps")
            for i, k in enumerate(evens):
                nc.tensor.matmul(
                    out=o31, lhsT=wo_sb[:, k, 128:256], rhs=ya3[:, k, :],
                    start=(i == 0), stop=False,
                )
            for k in odds:
                nc.gpsimd.dma_start(
                    out=ya3[:, k, :],
                    in_=y3_all[1][(k // 2) * 128 : (k // 2 + 1) * 128, :],
                )
            for i, k in enumerate(odds):
                nc.tensor.matmul(
                    out=o30, lhsT=wo_sb[:, k, 0:128], rhs=ya3[:, k, :],
                    start=False, stop=(i == 3),
                )
                nc.tensor.matmul(
                    out=o31, lhsT=wo_sb[:, k, 128:256], rhs=ya3[:, k, :],
                    start=False, stop=(i == 3),
                )
            for mb, o_ps in ((0, o30), (1, o31)):
                ob = ob_pool.tile([128, TQ], F32, name="ob")
                nc.vector.tensor_scalar_add(
                    out=ob, in0=o_ps, scalar1=bo_sb[:, mb : mb + 1]
                )
                nc.gpsimd.dma_start(
                    out=out.ap()[mb * 128 : (mb + 1) * 128, 3 * TQ :], in_=ob
                )
    nc.finalize()
    return nc


_NC = None


def _get_nc():
    global _NC
    if _NC is None:
        _NC = build_nc()
    return _NC


def _relay(w):
    # [KT*128, M] -> [128, KT*M] so each partition's DMA line is contiguous
    kt, m = w.shape[0] // 128, w.shape[1]
    return np.ascontiguousarray(
        w.reshape(kt, 128, m).transpose(1, 0, 2).reshape(128, kt * m).astype(BF_NP)
    )


def _relay_x(xb):
    # x^T [C, T] -> [128, NQ*KT*TQ] n-major so each 512-col block is one
    # contiguous-per-partition DMA
    xt = xb.T.reshape(KT, 128, NQ, TQ)
    return np.ascontiguousarray(
        xt.transpose(1, 2, 0, 3).reshape(128, NQ * KT * TQ).astype(BF_NP)
    )


def _in_maps(x, freqs_cos, freqs_sin, Wqkv, bqkv, Wproj, bproj):
    x = np.asarray(x, np.float32)
    Wqkv = np.asarray(Wqkv, np.float32)
    bqkv = np.asarray(bqkv, np.float32)
    Wproj = np.asarray(Wproj, np.float32)
    bproj = np.asarray(bproj, np.float32)
    cosT = np.asarray(freqs_cos, np.float32).T  # [32, T]
    sinT = np.asarray(freqs_sin, np.float32).T
    cos4 = np.ascontiguousarray(np.tile(cosT, (4, 1)).astype(BF_NP))
    # sinp rows (+s, -s) per 64-block: R = u*cos + swap(u*sinp)
    sin4 = np.ascontiguousarray(
        np.tile(np.concatenate([sinT, -sinT], axis=0), (2, 1)).astype(BF_NP)
    )
    perm = np.zeros((128, 128), np.float32)
    for j in range(128):
        i = j + 32 if (j % 64) < 32 else j - 32
        perm[i, j] = 1.0
    perm = perm.astype(BF_NP)
    tri = np.triu(np.ones((TK, TK), np.float32)).astype(BF_NP)
    sel = np.zeros((2, 128), np.float32)
    sel[0, 0:64] = 1.0
    sel[1, 64:128] = 1.0
    sel = sel.astype(BF_NP)
    bproj_eff = bproj + bqkv[2 * C : 3 * C] @ Wproj
    maps = []
    for r in range(NCORES):
        b, g = r // GROUPS, r % GROUPS
        sl = slice(DGRP * g, DGRP * (g + 1))
        maps.append(
            {
                "xr": _relay_x(x[b]),
                "wqr": _relay(Wqkv[:, 0 * C :][:, sl]),
                "wkr": _relay(Wqkv[:, 1 * C :][:, sl]),
                "wvr": _relay(Wqkv[:, 2 * C :][:, sl]),
                "wor": _relay(Wproj[:, sl]),
                "cos4": cos4,
                "sin4": sin4,
                "perm": perm,
                "tri": tri,
                "sel": sel,
                "bq": np.ascontiguousarray(bqkv[0 * C : 1 * C][sl]).reshape(DGRP, 1),
                "bk": np.ascontiguousarray(bqkv[1 * C : 2 * C][sl]).reshape(DGRP, 1),
                "bo": np.ascontiguousarray(bproj_eff[sl]).reshape(DGRP, 1),
            }
        )
    return maps


def _assemble(results):
    y = np.empty((B, T, C), np.float32)
    for b in range(B):
        cat = np.concatenate(
            [np.asarray(results[GROUPS * b + g]["out"]) for g in range(GROUPS)], axis=0
        )
        y[b] = cat.T
    return y


def kernel(**inputs):
    nc = _get_nc()
    res = run_bass_kernel_spmd(nc, _in_maps(**inputs), core_ids=list(range(NCORES)))
    return _assemble(res.results)


def kernel_traced(**inputs):
    import tempfile

    nc = _get_nc()
    tmpdir = tempfile.mkdtemp(prefix="mha_trace_")
    res = run_bass_kernel_spmd(
        nc,
        _in_maps(**inputs),
        core_ids=list(range(NCORES)),
        trace=True,
        trace_cores=list(range(NCORES)),
        tmpdir=tmpdir,
    )
    return _assemble(res.results), res.exec_time_ns, tmpdir
